# revision 16
# baseline (speedup 1.0000x reference)
"""Causal self-attention (B=2, T=2048, E=1024, 16 heads) on 8 TRN2 NeuronCores.

Sharding (Megatron-style, zero device-side collectives):
  core c in 0..7 -> batch b = c//4, head group hg = c%4 (4 heads, 256 head-dims).
  Each core computes, for its batch and its 4 heads:
    qT/kT = (w_q|w_k)^T x^T   (transposed layout: [head_dim, T], fp16)
    v     = x w_v             (natural layout: [T, head_dim] + ones column, fp16)
    sT    = kT^T-block matmuls -> [tk, tq] score blocks, causal-trimmed:
            for the diagonal 512x512 block of each chunk, only the q-columns
            at-or-after the key tile are computed (per-j column restriction),
            so only one 128x128 triangular mask is ever needed.
    expS  = exp(sT/8) (ACT, fp16 out) * tri-mask on the diagonal 128-block
    yT    = v_plus^T @ expS  -> [65, tq] psum; row 64 = softmax row-sums
    y_norm= yT[0:64] * broadcast(1/rowsum)
    out_c = y_norm^T w_proj[rows of its heads] -> partial [T, E] in fp16
  Host: out[b] = sum of the 4 fp16 partials (f32 accum) + b_proj + b_v @ w_proj.
  b_k is dropped (softmax invariant to per-row constants); b_q applied on-chip.

All matmul operands are fp16 (PSUM accumulation in f32); the x/weight HBM
stream and the output partials are fp16, halving HBM traffic vs f32.
"""

import numpy as np

N_HEAD = 16
E = 1024
B, T = 2, 2048
HD = E // N_HEAD          # 64
N_CORES = 8
HPC = 4                   # heads per core
DJ = HPC * HD             # 256 head-dim columns per core
ET = E // 128             # 8  e-tiles
TT = T // 128             # 16 t-tiles
TC = T // 512             # 4  t-chunks
SCALE = 1.0 / np.sqrt(HD)  # 0.125

_STATE = {}


def _build_nc(reps=1):
    import concourse.tile as tile
    from concourse import mybir
    from concourse.bacc import Bacc

    f32 = mybir.dt.float32
    f16 = mybir.dt.float16
    AF = mybir.ActivationFunctionType

    nc = Bacc()
    xT_d = nc.dram_tensor("xT", [E, T], f16, kind="ExternalInput")
    wqk_d = nc.dram_tensor("wqk", [E, 2 * DJ], f16, kind="ExternalInput")
    wv_d = nc.dram_tensor("wv", [E, DJ], f16, kind="ExternalInput")
    wp_d = nc.dram_tensor("wp", [DJ, E], f16, kind="ExternalInput")
    bq_d = nc.dram_tensor("bq", [128, 2], f32, kind="ExternalInput")
    tri_d = nc.dram_tensor("tri", [128, 2, 128], f16, kind="ExternalInput")
    out_d = nc.dram_tensor("out", [T, E], f16, kind="ExternalOutput")

    with tile.TileContext(nc) as tc:
        with (
            tc.tile_pool(name="xw", bufs=1) as xw,          # persistent inputs
            tc.tile_pool(name="qkv", bufs=1) as qkv,        # persistent qT/kT/v/yT
            tc.tile_pool(name="es", bufs=8) as esp,         # exp(score) blocks
            tc.tile_pool(name="nrm", bufs=3) as nrm,        # norm scratch
            tc.tile_pool(name="ob", bufs=3) as obp,         # output staging
            tc.tile_pool(name="ps", bufs=2, space="PSUM") as ps,
            tc.tile_pool(name="psy", bufs=2, space="PSUM") as psy,
        ):
          nmask = 0  # round-robin the diagonal mask-multiplies DVE/GPSIMD
          for _rep in range(reps):
            ones4_sb = xw.tile([128, HPC, 1], f16, tag="ones4", name="ones4")
            nc.vector.memset(ones4_sb[:], 1.0)

            # ---- single DMA stream (sync queue), strictly in first-use
            # order: (wv, x chunk0, wqk) interleaved per e-tile so chunk-0
            # v and q/k accumulations stream as tiles arrive; then x chunk 1
            # (needed by the fills inside attention chunk 0), then the small
            # tensors, then x chunks 2-3 ----
            wv_sb = []
            xT_sb = []
            wqk_sb = []
            for et in range(ET):
                t = xw.tile([128, DJ], f16, tag=f"wv{et}", name=f"wv{et}")
                nc.sync.dma_start(t[:], wv_d[128 * et : 128 * (et + 1), :])
                wv_sb.append(t)
                x = xw.tile([128, T], f16, tag=f"xT{et}", name=f"xT{et}")
                nc.sync.dma_start(x[:, 0:512], xT_d[128 * et : 128 * (et + 1), 0:512])
                xT_sb.append(x)
            for et in range(ET):
                t = xw.tile([128, 2 * DJ], f16, tag=f"wqk{et}", name=f"wqk{et}")
                nc.sync.dma_start(t[:], wqk_d[128 * et : 128 * (et + 1), :])
                wqk_sb.append(t)
            for et in range(ET):
                nc.sync.dma_start(
                    xT_sb[et][:, 512:1024],
                    xT_d[128 * et : 128 * (et + 1), 512:1024],
                )
            bq_sb = xw.tile([128, 2], f32, tag="bq", name="bq")
            nc.sync.dma_start(bq_sb[:], bq_d[:])
            tri_sb = xw.tile([128, 2, 128], f16, tag="tri", name="tri")
            nc.sync.dma_start(tri_sb[:], tri_d[:])
            wp_sb = []
            for kt in range(2):
                t = xw.tile([128, E], f16, tag=f"wp{kt}", name=f"wp{kt}")
                nc.sync.dma_start(t[:], wp_d[128 * kt : 128 * (kt + 1), :])
                wp_sb.append(t)
            for ci in range(2, TC):
                for et in range(ET):
                    nc.sync.dma_start(
                        xT_sb[et][:, 512 * ci : 512 * (ci + 1)],
                        xT_d[128 * et : 128 * (et + 1), 512 * ci : 512 * (ci + 1)],
                    )

            if reps > 1 and _rep > 0:
                # measurement builds: serialize reps by folding a read-back
                # sampling EVERY output t-tile of the previous rep into the
                # v ones-column (timing-only perturbation of ~1e-7)
                chain = xw.tile([128, TT, 4], f16, tag="chain", name="chain")
                nc.sync.dma_start(
                    chain[:],
                    out_d.rearrange("(n p) e -> p n e", p=128)[:, :, 0:4],
                )
                red = xw.tile([128, 1], f32, tag="red", name="red")
                nc.vector.tensor_reduce(
                    out=red[:], in_=chain[:], axis=mybir.AxisListType.XY,
                    op=mybir.AluOpType.add,
                )
                o4b = xw.tile([128, HPC, 1], f16, tag="ones4b", name="ones4b")
                rs = xw.tile([128, 1], f32, tag="rs", name="rs")
                nc.vector.tensor_scalar_mul(rs[:], red[:], 1e-7)
                with nc.allow_low_precision(reason="timing chain"):
                    nc.vector.tensor_scalar_add(o4b[:], ones4_sb[:], rs[:])
                ones4_sb = o4b

            # persistent intermediates (all fp16)
            qT_sb = [qkv.tile([128, T], f16, tag=f"qT{i}", name=f"qT{i}") for i in range(2)]
            kT_sb = [qkv.tile([128, T], f16, tag=f"kT{i}", name=f"kT{i}") for i in range(2)]
            v_sb = [qkv.tile([128, HPC, HD + 1], f16, tag=f"v{i}", name=f"v{i}") for i in range(TT)]
            yT_sb = [qkv.tile([128, T], f16, tag=f"yT{i}", name=f"yT{i}") for i in range(2)]

            # ---- emission helpers (PE is in-order: the emitted sequence IS
            # the PE execution order, so attention score/PV chains are
            # software-pipelined and padded with independent "fill" groups
            # from neighboring chunks' qkv/projection work) ----
            def v_group(tt):
                acc = ps.tile([128, DJ], f32, tag="mm", name="acc_v")
                for et in range(ET):
                    nc.tensor.matmul(
                        acc[:],
                        xT_sb[et][:, 128 * tt : 128 * (tt + 1)],
                        wv_sb[et][:],
                        start=(et == 0),
                        stop=(et == ET - 1),
                    )
                nc.vector.tensor_copy(
                    v_sb[tt][:, :, 0:HD],
                    acc[:].rearrange("p (h d) -> p h d", h=HPC),
                )
                nc.vector.tensor_copy(v_sb[tt][:, :, HD : HD + 1], ones4_sb[:])

            def qk_group(ci, jt):      # jt 0,1 -> q ; 2,3 -> k
                acc = ps.tile([128, 512], f32, tag="mm", name="acc_qk")
                for et in range(ET):
                    nc.tensor.matmul(
                        acc[:],
                        wqk_sb[et][:, 128 * jt : 128 * (jt + 1)],
                        xT_sb[et][:, 512 * ci : 512 * (ci + 1)],
                        start=(et == 0),
                        stop=(et == ET - 1),
                    )
                if jt < 2:
                    nc.vector.tensor_scalar_add(
                        qT_sb[jt][:, 512 * ci : 512 * (ci + 1)],
                        acc[:],
                        bq_sb[:, jt : jt + 1],
                    )
                else:
                    nc.vector.tensor_copy(
                        kT_sb[jt - 2][:, 512 * ci : 512 * (ci + 1)], acc[:]
                    )

            ob_tiles = {}

            def proj_half(tt, nk, on_act=False):
                if nk == 0:
                    ob_tiles[tt] = obp.tile([128, E], f16, tag="ob", name="ob")
                ob = ob_tiles[tt]
                acc = ps.tile([128, 512], f32, tag="mm", name="acc_p")
                for kt in range(2):
                    nc.tensor.matmul(
                        acc[:],
                        yT_sb[kt][:, 128 * tt : 128 * (tt + 1)],
                        wp_sb[kt][:, 512 * nk : 512 * (nk + 1)],
                        start=(kt == 0),
                        stop=(kt == 1),
                    )
                if on_act:
                    # tail: DVE is saturated by the drain, ACT is idle
                    nc.scalar.activation(
                        out=ob[:, 512 * nk : 512 * (nk + 1)], in_=acc[:],
                        func=AF.Copy,
                    )
                else:
                    nc.vector.tensor_copy(ob[:, 512 * nk : 512 * (nk + 1)], acc[:])
                nc.sync.dma_start(
                    out_d[128 * tt : 128 * (tt + 1), 512 * nk : 512 * (nk + 1)],
                    ob[:, 512 * nk : 512 * (nk + 1)],
                )

            fills = []                 # FIFO of pending fill groups

            def emit_fill():
                if fills:
                    fills.pop(0)()

            # ---- chunk-0 qkv: v first (only needs wv + x chunk 0), then
            # the q/k tiles pair 0 needs; pair-1's q/k become fill work ----
            for tt in range(4):
                v_group(tt)
            qk_group(0, 0)
            qk_group(0, 2)
            fills.append(lambda: qk_group(0, 1))
            fills.append(lambda: qk_group(0, 3))

            # ---- attention chunks, each interleaved with the NEXT chunk's
            # qkv groups and the PREVIOUS chunk's projection as fills ----
            for ci in range(TC):
                if ci + 1 < TC:
                    for tt in range(4 * ci + 4, 4 * ci + 8):
                        fills.append(lambda tt=tt: v_group(tt))
                    for jt in (0, 2, 1, 3):
                        fills.append(lambda c1=ci + 1, jt=jt: qk_group(c1, jt))
                else:
                    # last chunk: all deferred projections (chunks 0..2)
                    # become its fill work (it has the longest, otherwise
                    # ACT-bound score/exp chain)
                    for tt in range(4 * ci):
                        for nk in range(2):
                            fills.append(lambda tt=tt, nk=nk: proj_half(tt, nk))

                nj = 4 * ci + 4
                # late-biased fill rationing: when there are fewer fills
                # than slots, leave the EARLY slots empty — the score/exp
                # chain only falls behind the PE near the chunk tail
                skip = max(0, 2 * nj - len(fills))
                for hp in range(2):           # head pair: heads 2hp, 2hp+1
                    kth = kT_sb[hp]
                    qth = qT_sb[hp]
                    ya = psy.tile([HD + 1, 512], f32, tag="y", name="ya")
                    yb = psy.tile([HD + 1, 512], f32, tag="y", name="yb")

                    def scores(j):
                        m = j - 4 * ci
                        col0 = 128 * m if m > 0 else 0
                        s2 = ps.tile([128, 1024], f32, tag="s2", name="s2")
                        for half in range(2):
                            nc.tensor.matmul(
                                s2[:, 512 * half + col0 : 512 * half + 512],
                                kth[HD * half : HD * half + HD,
                                    128 * j : 128 * (j + 1)],
                                qth[HD * half : HD * half + HD,
                                    512 * ci + col0 : 512 * (ci + 1)],
                            )
                        es = esp.tile([128, 1024], f16, tag="es", name="es")
                        if col0 == 0:
                            nc.scalar.activation(
                                out=es[:], in_=s2[:], func=AF.Exp,
                                scale=float(SCALE),
                            )
                        else:
                            s2v = s2[:].rearrange("p (h q) -> p h q", h=2)[:, :, col0:512]
                            esv = es[:].rearrange("p (h q) -> p h q", h=2)[:, :, col0:512]
                            nc.scalar.activation(
                                out=esv, in_=s2v, func=AF.Exp, scale=float(SCALE)
                            )
                        if m >= 0:
                            # triangular mask on the exact-diagonal 128-block
                            # of both heads (strided 3D view, one multiply)
                            esm = es[:].rearrange("p (h q) -> p h q", h=2)[
                                :, :, col0 : col0 + 128
                            ]
                            nc.gpsimd.tensor_mul(esm, esm, tri_sb[:])
                        return es, col0

                    es_q = [scores(0)]
                    for j in range(nj):
                        if j + 1 < nj:
                            es_q.append(scores(j + 1))
                        if skip > 0:
                            skip -= 1
                        else:
                            emit_fill()
                        es, col0 = es_q.pop(0)
                        nc.tensor.matmul(
                            ya[:, col0:512], v_sb[j][:, 2 * hp, :],
                            es[:, col0:512],
                            start=(j == 0), stop=(j == nj - 1),
                            skip_group_check=True,
                        )
                        nc.tensor.matmul(
                            yb[:, col0:512], v_sb[j][:, 2 * hp + 1, :],
                            es[:, 512 + col0 : 1024],
                            start=(j == 0), stop=(j == nj - 1),
                            skip_group_check=True,
                        )
                    # normalize: evacuate psum fast (fp16 copy frees the
                    # bank for the next head pair), then recip/broadcast/mul
                    # entirely in fp16 SBUF
                    for half, yy in ((0, ya), (1, yb)):
                        y16 = nrm.tile([HD + 1, 512], f16, tag="y16", name="y16")
                        if ci == TC - 1:
                            nc.scalar.activation(out=y16[:], in_=yy[:], func=AF.Copy)
                        else:
                            nc.vector.tensor_copy(y16[:], yy[:])
                        rrow = nrm.tile([1, 512], f16, tag="rr", name="rrow")
                        with nc.allow_low_precision(reason="fp16 softmax recip"):
                            nc.vector.reciprocal(rrow[:], y16[HD : HD + 1, :])
                        bs = nrm.tile([HD, 512], f16, tag="bs", name="bs")
                        nc.gpsimd.partition_broadcast(bs[:], rrow[:])
                        nc.vector.tensor_mul(
                            yT_sb[hp][HD * half : HD * half + HD,
                                      512 * ci : 512 * (ci + 1)],
                            y16[0:HD, :],
                            bs[:],
                        )

            # drain remaining fills, then the last chunk's projection
            # (psum-evacuation on the now-idle ACT engine)
            while fills:
                emit_fill()
            for tt in range(T // 128 - 4, T // 128):
                for nk in range(2):
                    proj_half(tt, nk, on_act=True)

    nc.finalize()
    return nc


def _host_constants():
    # duplicated [128, 2, 128] upper-triangular mask: tri[r, :, c] = (c >= r)
    r = np.arange(128)[:, None]
    c = np.arange(128)[None, :]
    tri = (c >= r).astype(np.float16)
    return np.ascontiguousarray(np.broadcast_to(tri[:, None, :], (128, 2, 128)))


def _make_in_maps(x, w_qkv, b_qkv):
    tri = _host_constants()
    in_maps = []
    for c in range(N_CORES):
        b, hg = divmod(c, HPC)
        j0 = DJ * hg
        xT = np.ascontiguousarray(
            np.asarray(x[b], dtype=np.float32).T.astype(np.float16)
        )
        wq = w_qkv[:, j0 : j0 + DJ]
        wk = w_qkv[:, E + j0 : E + j0 + DJ]
        wqk = np.concatenate([wq, wk], axis=1).astype(np.float16)
        wv = np.ascontiguousarray(
            np.asarray(w_qkv[:, 2 * E + j0 : 2 * E + j0 + DJ], dtype=np.float32)
        ).astype(np.float16)
        bq = np.ascontiguousarray(
            np.asarray(b_qkv[j0 : j0 + DJ], dtype=np.float32).reshape(2, 128).T
        )
        in_maps.append(
            {
                "xT": xT,
                "wqk": np.ascontiguousarray(wqk),
                "wv": wv,
                "wp": None,  # filled in kernel() (needs w_proj)
                "bq": bq,
                "tri": tri,
            }
        )
    return in_maps


def _get_exec():
    """Build the Bass module and a cached jitted SPMD callable (once)."""
    if "exec" in _STATE:
        return _STATE["exec"]

    import jax
    from concourse import bass2jax, mybir
    from jax.experimental.shard_map import shard_map
    from jax.sharding import Mesh, PartitionSpec

    nc = _build_nc()
    _STATE["nc"] = nc
    bass2jax.install_neuronx_cc_hook()

    partition_name = (
        nc.partition_id_tensor.name if nc.partition_id_tensor else None
    )
    in_names = []
    out_names = []
    out_avals = []
    zero_outs = []
    for alloc in nc.m.functions[0].allocations:
        if not isinstance(alloc, mybir.MemoryLocationSet):
            continue
        name = alloc.memorylocations[0].name
        if alloc.kind == "ExternalInput":
            if name != partition_name:
                in_names.append(name)
        elif alloc.kind == "ExternalOutput":
            shape = tuple(alloc.tensor_shape)
            dtype = mybir.dt.np(alloc.dtype)
            out_names.append(name)
            out_avals.append(jax.core.ShapedArray(shape, dtype))
            zero_outs.append(np.zeros(shape, dtype))
    n_params = len(in_names)
    all_names = in_names + out_names
    if partition_name is not None:
        all_names = all_names + [partition_name]

    def _make_body(k):
        def _body(*args):
            operands = list(args)
            if partition_name is not None:
                operands.append(bass2jax.partition_id_tensor())
            for _ in range(k):
                outs = bass2jax._bass_exec_p.bind(
                    *operands,
                    out_avals=tuple(out_avals),
                    in_names=tuple(all_names),
                    out_names=tuple(out_names),
                    lowering_input_output_aliases=(),
                    sim_require_finite=True,
                    sim_require_nnan=True,
                    nc=nc,
                )
            return tuple(outs)

        return _body

    devices = jax.devices()[:N_CORES]
    mesh = Mesh(np.asarray(devices), ("core",))
    n_all = n_params + len(out_names)

    def _make_sharded(k):
        return jax.jit(
            shard_map(
                _make_body(k),
                mesh=mesh,
                in_specs=(PartitionSpec("core"),) * n_all,
                out_specs=(PartitionSpec("core"),) * len(out_names),
                check_rep=False,
            ),
            keep_unused=True,
        )

    sharded = _make_sharded(1)

    state = {
        "make_sharded": _make_sharded,
        "jax": jax,
        "sharded": sharded,
        "in_names": in_names,
        "out_names": out_names,
        "out_avals": out_avals,
        "zeros_dev": [
            jax.device_put(
                np.zeros((N_CORES * z.shape[0], *z.shape[1:]), z.dtype)
            )
            for z in zero_outs
        ],
    }
    _STATE["exec"] = state
    return state


def _concat_inputs(in_maps):
    st = _get_exec()
    return [
        np.concatenate([np.asarray(in_maps[c][name]) for c in range(N_CORES)], axis=0)
        for name in st["in_names"]
    ]


def _run_device(concat_in):
    """concat_in: list of global (8*dim0, ...) arrays (np or jax). Returns
    list of per-core output dicts."""
    st = _get_exec()
    out_arrs = st["sharded"](*concat_in, *st["zeros_dev"])
    res = []
    for c in range(N_CORES):
        d = {}
        for i, name in enumerate(st["out_names"]):
            shp = st["out_avals"][i].shape
            d[name] = np.asarray(out_arrs[i]).reshape(N_CORES, *shp)[c]
        res.append(d)
    return res


def kernel(x, w_qkv, b_qkv, w_proj, b_proj):
    x = np.asarray(x, dtype=np.float32)
    w_qkv = np.asarray(w_qkv, dtype=np.float32)
    b_qkv = np.asarray(b_qkv, dtype=np.float32)
    w_proj = np.asarray(w_proj, dtype=np.float32)
    b_proj = np.asarray(b_proj, dtype=np.float32)

    in_maps = _make_in_maps(x, w_qkv, b_qkv)
    for c in range(N_CORES):
        _, hg = divmod(c, HPC)
        j0 = DJ * hg
        in_maps[c]["wp"] = np.ascontiguousarray(
            w_proj[j0 : j0 + DJ, :].astype(np.float16)
        )

    results = _run_device(_concat_inputs(in_maps))

    out = np.zeros((B, T, E), dtype=np.float32)
    for c in range(N_CORES):
        out[c // HPC] += np.asarray(results[c]["out"], dtype=np.float32)
    # fold b_v through the projection; b_k cancels inside softmax
    bias = b_proj + b_qkv[2 * E :] @ w_proj
    out += bias[None, None, :]
    return out


# revision 19
# speedup vs baseline: 17.9917x; 17.9917x over previous
"""Causal self-attention (B=2, T=2048, E=1024, 16 heads) on 8 TRN2 NeuronCores.

Sharding (Megatron-style, zero device-side collectives):
  core c in 0..7 -> batch b = c//4, head group hg = c%4 (4 heads, 256 head-dims).
  Each core computes, for its batch and its 4 heads:
    qT/kT = (w_q|w_k)^T x^T   (transposed layout: [head_dim, T], fp16)
    v     = x w_v             (natural layout: [T, head_dim] + ones column, fp16)
    sT    = kT^T-block matmuls -> [tk, tq] score blocks, causal-trimmed:
            for the diagonal 512x512 block of each chunk, only the q-columns
            at-or-after the key tile are computed (per-j column restriction),
            so only one 128x128 triangular mask is ever needed.
    expS  = exp(sT/8) (ACT, fp16 out) * tri-mask on the diagonal 128-block
    yT    = v_plus^T @ expS  -> [65, tq] psum; row 64 = softmax row-sums
    y_norm= yT[0:64] * broadcast(1/rowsum)
    out_c = y_norm^T w_proj[rows of its heads] -> partial [T, E] in fp16
  Host: out[b] = sum of the 4 fp16 partials (f32 accum) + b_proj + b_v @ w_proj.
  b_k is dropped (softmax invariant to per-row constants); b_q applied on-chip.

All matmul operands are fp16 (PSUM accumulation in f32); the x/weight HBM
stream and the output partials are fp16, halving HBM traffic vs f32.
"""

import os

import numpy as np

N_HEAD = 16
E = 1024
B, T = 2, 2048
HD = E // N_HEAD          # 64
N_CORES = 8
HPC = 4                   # heads per core
DJ = HPC * HD             # 256 head-dim columns per core
ET = E // 128             # 8  e-tiles
TT = T // 128             # 16 t-tiles
TC = T // 512             # 4  t-chunks
SCALE = 1.0 / np.sqrt(HD)  # 0.125

_STATE = {}


def _build_nc(reps=1):
    import concourse.tile as tile
    from concourse import mybir
    from concourse.bacc import Bacc

    f32 = mybir.dt.float32
    f16 = mybir.dt.float16
    AF = mybir.ActivationFunctionType

    nc = Bacc()
    xT_d = nc.dram_tensor("xT", [E, T], f16, kind="ExternalInput")
    wqk_d = nc.dram_tensor("wqk", [E, 2 * DJ], f16, kind="ExternalInput")
    wv_d = nc.dram_tensor("wv", [E, DJ], f16, kind="ExternalInput")
    wp_d = nc.dram_tensor("wp", [DJ, E], f16, kind="ExternalInput")
    bq_d = nc.dram_tensor("bq", [128, 2], f32, kind="ExternalInput")
    tri_d = nc.dram_tensor("tri", [128, 2, 128], f16, kind="ExternalInput")
    out_d = nc.dram_tensor("out", [T, E], f16, kind="ExternalOutput")

    with tile.TileContext(nc) as tc:
        with (
            tc.tile_pool(name="xw", bufs=1) as xw,          # persistent inputs
            tc.tile_pool(name="qkv", bufs=1) as qkv,        # persistent qT/kT/v/yT
            tc.tile_pool(name="es", bufs=8) as esp,         # exp(score) blocks
            tc.tile_pool(name="nrm", bufs=3) as nrm,        # norm scratch
            tc.tile_pool(name="ob", bufs=3) as obp,         # output staging
            tc.tile_pool(name="ps", bufs=2, space="PSUM") as ps,
            tc.tile_pool(name="psy", bufs=2, space="PSUM") as psy,
        ):
          nmask = 0  # round-robin the diagonal mask-multiplies DVE/GPSIMD
          for _rep in range(reps):
            ones4_sb = xw.tile([128, HPC, 1], f16, tag="ones4", name="ones4")
            nc.vector.memset(ones4_sb[:], 1.0)

            # ---- single DMA stream (sync queue), strictly in first-use
            # order: (wv, x chunk0, wqk) interleaved per e-tile so chunk-0
            # v and q/k accumulations stream as tiles arrive; then x chunk 1
            # (needed by the fills inside attention chunk 0), then the small
            # tensors, then x chunks 2-3 ----
            wv_sb = []
            xT_sb = []
            wqk_sb = []
            for et in range(ET):
                t = xw.tile([128, DJ], f16, tag=f"wv{et}", name=f"wv{et}")
                nc.sync.dma_start(t[:], wv_d[128 * et : 128 * (et + 1), :])
                wv_sb.append(t)
                x = xw.tile([128, T], f16, tag=f"xT{et}", name=f"xT{et}")
                nc.sync.dma_start(x[:, 0:512], xT_d[128 * et : 128 * (et + 1), 0:512])
                xT_sb.append(x)
            for et in range(ET):
                t = xw.tile([128, 2 * DJ], f16, tag=f"wqk{et}", name=f"wqk{et}")
                nc.sync.dma_start(t[:], wqk_d[128 * et : 128 * (et + 1), :])
                wqk_sb.append(t)
            for et in range(ET):
                nc.sync.dma_start(
                    xT_sb[et][:, 512:1024],
                    xT_d[128 * et : 128 * (et + 1), 512:1024],
                )
            bq_sb = xw.tile([128, 2], f32, tag="bq", name="bq")
            nc.sync.dma_start(bq_sb[:], bq_d[:])
            tri_sb = xw.tile([128, 2, 128], f16, tag="tri", name="tri")
            nc.sync.dma_start(tri_sb[:], tri_d[:])
            wp_sb = []
            for kt in range(2):
                t = xw.tile([128, E], f16, tag=f"wp{kt}", name=f"wp{kt}")
                nc.sync.dma_start(t[:], wp_d[128 * kt : 128 * (kt + 1), :])
                wp_sb.append(t)
            for ci in range(2, TC):
                for et in range(ET):
                    nc.sync.dma_start(
                        xT_sb[et][:, 512 * ci : 512 * (ci + 1)],
                        xT_d[128 * et : 128 * (et + 1), 512 * ci : 512 * (ci + 1)],
                    )

            if reps > 1 and _rep > 0:
                # measurement builds: serialize reps by folding a read-back
                # sampling EVERY output t-tile of the previous rep into the
                # v ones-column (timing-only perturbation of ~1e-7)
                chain = xw.tile([128, TT, 4], f16, tag="chain", name="chain")
                nc.sync.dma_start(
                    chain[:],
                    out_d.rearrange("(n p) e -> p n e", p=128)[:, :, 0:4],
                )
                red = xw.tile([128, 1], f32, tag="red", name="red")
                nc.vector.tensor_reduce(
                    out=red[:], in_=chain[:], axis=mybir.AxisListType.XY,
                    op=mybir.AluOpType.add,
                )
                o4b = xw.tile([128, HPC, 1], f16, tag="ones4b", name="ones4b")
                rs = xw.tile([128, 1], f32, tag="rs", name="rs")
                nc.vector.tensor_scalar_mul(rs[:], red[:], 1e-7)
                with nc.allow_low_precision(reason="timing chain"):
                    nc.vector.tensor_scalar_add(o4b[:], ones4_sb[:], rs[:])
                ones4_sb = o4b

            # persistent intermediates (all fp16)
            qT_sb = [qkv.tile([128, T], f16, tag=f"qT{i}", name=f"qT{i}") for i in range(2)]
            kT_sb = [qkv.tile([128, T], f16, tag=f"kT{i}", name=f"kT{i}") for i in range(2)]
            v_sb = [qkv.tile([128, HPC, HD + 1], f16, tag=f"v{i}", name=f"v{i}") for i in range(TT)]
            yT_sb = [qkv.tile([128, T], f16, tag=f"yT{i}", name=f"yT{i}") for i in range(2)]

            # ---- emission helpers (PE is in-order: the emitted sequence IS
            # the PE execution order, so attention score/PV chains are
            # software-pipelined and padded with independent "fill" groups
            # from neighboring chunks' qkv/projection work) ----
            def v_group(tt):
                acc = ps.tile([128, DJ], f32, tag="mm", name="acc_v")
                for et in range(ET):
                    nc.tensor.matmul(
                        acc[:],
                        xT_sb[et][:, 128 * tt : 128 * (tt + 1)],
                        wv_sb[et][:],
                        start=(et == 0),
                        stop=(et == ET - 1),
                    )
                nc.vector.tensor_copy(
                    v_sb[tt][:, :, 0:HD],
                    acc[:].rearrange("p (h d) -> p h d", h=HPC),
                )
                nc.vector.tensor_copy(v_sb[tt][:, :, HD : HD + 1], ones4_sb[:])

            def qk_group(ci, jt):      # jt 0,1 -> q ; 2,3 -> k
                acc = ps.tile([128, 512], f32, tag="mm", name="acc_qk")
                for et in range(ET):
                    nc.tensor.matmul(
                        acc[:],
                        wqk_sb[et][:, 128 * jt : 128 * (jt + 1)],
                        xT_sb[et][:, 512 * ci : 512 * (ci + 1)],
                        start=(et == 0),
                        stop=(et == ET - 1),
                    )
                if jt < 2:
                    nc.vector.tensor_scalar_add(
                        qT_sb[jt][:, 512 * ci : 512 * (ci + 1)],
                        acc[:],
                        bq_sb[:, jt : jt + 1],
                    )
                else:
                    nc.vector.tensor_copy(
                        kT_sb[jt - 2][:, 512 * ci : 512 * (ci + 1)], acc[:]
                    )

            ob_tiles = {}

            def proj_half(tt, nk, on_act=False):
                if nk == 0:
                    ob_tiles[tt] = obp.tile([128, E], f16, tag="ob", name="ob")
                ob = ob_tiles[tt]
                acc = ps.tile([128, 512], f32, tag="mm", name="acc_p")
                for kt in range(2):
                    nc.tensor.matmul(
                        acc[:],
                        yT_sb[kt][:, 128 * tt : 128 * (tt + 1)],
                        wp_sb[kt][:, 512 * nk : 512 * (nk + 1)],
                        start=(kt == 0),
                        stop=(kt == 1),
                    )
                if on_act:
                    # tail: DVE is saturated by the drain, ACT is idle
                    nc.scalar.activation(
                        out=ob[:, 512 * nk : 512 * (nk + 1)], in_=acc[:],
                        func=AF.Copy,
                    )
                else:
                    nc.vector.tensor_copy(ob[:, 512 * nk : 512 * (nk + 1)], acc[:])
                nc.sync.dma_start(
                    out_d[128 * tt : 128 * (tt + 1), 512 * nk : 512 * (nk + 1)],
                    ob[:, 512 * nk : 512 * (nk + 1)],
                )
                if os.environ.get("KERNEL_OUT_X2"):
                    # timing probe: double the output HBM traffic
                    nc.sync.dma_start(
                        out_d[128 * tt : 128 * (tt + 1),
                              512 * nk : 512 * (nk + 1)],
                        ob[:, 512 * nk : 512 * (nk + 1)],
                    )

            fills = []                 # FIFO of pending fill groups

            def emit_fill():
                if fills:
                    fills.pop(0)()

            # ---- chunk-0 qkv: v first (only needs wv + x chunk 0), then
            # the q/k tiles pair 0 needs; pair-1's q/k become fill work ----
            for tt in range(4):
                v_group(tt)
            qk_group(0, 0)
            qk_group(0, 2)
            fills.append(lambda: qk_group(0, 1))
            fills.append(lambda: qk_group(0, 3))

            # ---- attention chunks, each interleaved with the NEXT chunk's
            # qkv groups and the PREVIOUS chunk's projection as fills ----
            for ci in range(TC):
                if ci + 1 < TC:
                    for tt in range(4 * ci + 4, 4 * ci + 8):
                        fills.append(lambda tt=tt: v_group(tt))
                    for jt in (0, 2, 1, 3):
                        fills.append(lambda c1=ci + 1, jt=jt: qk_group(c1, jt))
                else:
                    # last chunk: all deferred projections (chunks 0..2)
                    # become its fill work (it has the longest, otherwise
                    # ACT-bound score/exp chain)
                    for tt in range(4 * ci):
                        for nk in range(2):
                            fills.append(lambda tt=tt, nk=nk: proj_half(tt, nk))

                nj = 4 * ci + 4
                # late-biased fill rationing: when there are fewer fills
                # than slots, leave the EARLY slots empty — the score/exp
                # chain only falls behind the PE near the chunk tail
                skip = max(0, 2 * nj - len(fills))
                for hp in range(2):           # head pair: heads 2hp, 2hp+1
                    kth = kT_sb[hp]
                    qth = qT_sb[hp]
                    ya = psy.tile([HD + 1, 512], f32, tag="y", name="ya")
                    yb = psy.tile([HD + 1, 512], f32, tag="y", name="yb")

                    def scores(j):
                        m = j - 4 * ci
                        col0 = 128 * m if m > 0 else 0
                        s2 = ps.tile([128, 1024], f32, tag="s2", name="s2")
                        for half in range(2):
                            nc.tensor.matmul(
                                s2[:, 512 * half + col0 : 512 * half + 512],
                                kth[HD * half : HD * half + HD,
                                    128 * j : 128 * (j + 1)],
                                qth[HD * half : HD * half + HD,
                                    512 * ci + col0 : 512 * (ci + 1)],
                            )
                        if os.environ.get("KERNEL_MM_X2"):
                            # timing probe: double the score matmuls
                            for half in range(2):
                                nc.tensor.matmul(
                                    s2[:, 512 * half + col0 : 512 * half + 512],
                                    kth[HD * half : HD * half + HD,
                                        128 * j : 128 * (j + 1)],
                                    qth[HD * half : HD * half + HD,
                                        512 * ci + col0 : 512 * (ci + 1)],
                                )
                        es = esp.tile([128, 1024], f16, tag="es", name="es")
                        if col0 == 0:
                            nc.scalar.activation(
                                out=es[:], in_=s2[:], func=AF.Exp,
                                scale=float(SCALE),
                            )
                        else:
                            s2v = s2[:].rearrange("p (h q) -> p h q", h=2)[:, :, col0:512]
                            esv = es[:].rearrange("p (h q) -> p h q", h=2)[:, :, col0:512]
                            nc.scalar.activation(
                                out=esv, in_=s2v, func=AF.Exp, scale=float(SCALE)
                            )
                        if os.environ.get("KERNEL_EXP_X2"):
                            # timing probe: double the ACT exp work
                            es2 = esp.tile([128, 1024], f16, tag="esx",
                                           name="esx", bufs=2)
                            nc.scalar.activation(
                                out=es2[:, col0:1024], in_=s2[:, col0:1024],
                                func=AF.Exp, scale=float(SCALE),
                            )
                        if m >= 0:
                            # triangular mask on the exact-diagonal 128-block
                            # of both heads (strided 3D view, one multiply)
                            esm = es[:].rearrange("p (h q) -> p h q", h=2)[
                                :, :, col0 : col0 + 128
                            ]
                            nc.gpsimd.tensor_mul(esm, esm, tri_sb[:])
                        return es, col0

                    es_q = [scores(0)]
                    for j in range(nj):
                        if j + 1 < nj:
                            es_q.append(scores(j + 1))
                        if skip > 0:
                            skip -= 1
                        else:
                            emit_fill()
                        es, col0 = es_q.pop(0)
                        nc.tensor.matmul(
                            ya[:, col0:512], v_sb[j][:, 2 * hp, :],
                            es[:, col0:512],
                            start=(j == 0), stop=(j == nj - 1),
                            skip_group_check=True,
                        )
                        nc.tensor.matmul(
                            yb[:, col0:512], v_sb[j][:, 2 * hp + 1, :],
                            es[:, 512 + col0 : 1024],
                            start=(j == 0), stop=(j == nj - 1),
                            skip_group_check=True,
                        )
                    # normalize: evacuate psum fast (fp16 copy frees the
                    # bank for the next head pair), then recip/broadcast/mul
                    # entirely in fp16 SBUF
                    for half, yy in ((0, ya), (1, yb)):
                        y16 = nrm.tile([HD + 1, 512], f16, tag="y16", name="y16")
                        if ci == TC - 1:
                            nc.scalar.activation(out=y16[:], in_=yy[:], func=AF.Copy)
                        else:
                            nc.vector.tensor_copy(y16[:], yy[:])
                        rrow = nrm.tile([1, 512], f16, tag="rr", name="rrow")
                        with nc.allow_low_precision(reason="fp16 softmax recip"):
                            nc.vector.reciprocal(rrow[:], y16[HD : HD + 1, :])
                        bs = nrm.tile([HD, 512], f16, tag="bs", name="bs")
                        nc.gpsimd.partition_broadcast(bs[:], rrow[:])
                        nc.vector.tensor_mul(
                            yT_sb[hp][HD * half : HD * half + HD,
                                      512 * ci : 512 * (ci + 1)],
                            y16[0:HD, :],
                            bs[:],
                        )

            # drain remaining fills, then the last chunk's projection
            # (psum-evacuation on the now-idle ACT engine)
            while fills:
                emit_fill()
            for tt in range(T // 128 - 4, T // 128):
                for nk in range(2):
                    proj_half(tt, nk, on_act=True)

    nc.finalize()
    return nc


def _host_constants():
    # duplicated [128, 2, 128] upper-triangular mask: tri[r, :, c] = (c >= r)
    r = np.arange(128)[:, None]
    c = np.arange(128)[None, :]
    tri = (c >= r).astype(np.float16)
    return np.ascontiguousarray(np.broadcast_to(tri[:, None, :], (128, 2, 128)))


def _make_in_maps(x, w_qkv, b_qkv):
    tri = _host_constants()
    in_maps = []
    for c in range(N_CORES):
        b, hg = divmod(c, HPC)
        j0 = DJ * hg
        xT = np.ascontiguousarray(
            np.asarray(x[b], dtype=np.float32).T.astype(np.float16)
        )
        wq = w_qkv[:, j0 : j0 + DJ]
        wk = w_qkv[:, E + j0 : E + j0 + DJ]
        wqk = np.concatenate([wq, wk], axis=1).astype(np.float16)
        wv = np.ascontiguousarray(
            np.asarray(w_qkv[:, 2 * E + j0 : 2 * E + j0 + DJ], dtype=np.float32)
        ).astype(np.float16)
        bq = np.ascontiguousarray(
            np.asarray(b_qkv[j0 : j0 + DJ], dtype=np.float32).reshape(2, 128).T
        )
        in_maps.append(
            {
                "xT": xT,
                "wqk": np.ascontiguousarray(wqk),
                "wv": wv,
                "wp": None,  # filled in kernel() (needs w_proj)
                "bq": bq,
                "tri": tri,
            }
        )
    return in_maps


def _get_exec():
    """Build the Bass module and a cached jitted SPMD callable (once)."""
    if "exec" in _STATE:
        return _STATE["exec"]

    import jax
    from concourse import bass2jax, mybir
    from jax.experimental.shard_map import shard_map
    from jax.sharding import Mesh, PartitionSpec

    nc = _build_nc()
    _STATE["nc"] = nc
    bass2jax.install_neuronx_cc_hook()

    partition_name = (
        nc.partition_id_tensor.name if nc.partition_id_tensor else None
    )
    in_names = []
    out_names = []
    out_avals = []
    zero_outs = []
    for alloc in nc.m.functions[0].allocations:
        if not isinstance(alloc, mybir.MemoryLocationSet):
            continue
        name = alloc.memorylocations[0].name
        if alloc.kind == "ExternalInput":
            if name != partition_name:
                in_names.append(name)
        elif alloc.kind == "ExternalOutput":
            shape = tuple(alloc.tensor_shape)
            dtype = mybir.dt.np(alloc.dtype)
            out_names.append(name)
            out_avals.append(jax.core.ShapedArray(shape, dtype))
            zero_outs.append(np.zeros(shape, dtype))
    n_params = len(in_names)
    all_names = in_names + out_names
    if partition_name is not None:
        all_names = all_names + [partition_name]

    def _make_body(k):
        def _body(*args):
            operands = list(args)
            if partition_name is not None:
                operands.append(bass2jax.partition_id_tensor())
            for _ in range(k):
                outs = bass2jax._bass_exec_p.bind(
                    *operands,
                    out_avals=tuple(out_avals),
                    in_names=tuple(all_names),
                    out_names=tuple(out_names),
                    lowering_input_output_aliases=(),
                    sim_require_finite=True,
                    sim_require_nnan=True,
                    nc=nc,
                )
            return tuple(outs)

        return _body

    devices = jax.devices()[:N_CORES]
    mesh = Mesh(np.asarray(devices), ("core",))
    n_all = n_params + len(out_names)

    def _make_sharded(k):
        return jax.jit(
            shard_map(
                _make_body(k),
                mesh=mesh,
                in_specs=(PartitionSpec("core"),) * n_all,
                out_specs=(PartitionSpec("core"),) * len(out_names),
                check_rep=False,
            ),
            keep_unused=True,
        )

    sharded = _make_sharded(1)

    state = {
        "make_sharded": _make_sharded,
        "jax": jax,
        "sharded": sharded,
        "in_names": in_names,
        "out_names": out_names,
        "out_avals": out_avals,
        "zeros_dev": [
            jax.device_put(
                np.zeros((N_CORES * z.shape[0], *z.shape[1:]), z.dtype)
            )
            for z in zero_outs
        ],
    }
    _STATE["exec"] = state
    return state


def _concat_inputs(in_maps):
    st = _get_exec()
    return [
        np.concatenate([np.asarray(in_maps[c][name]) for c in range(N_CORES)], axis=0)
        for name in st["in_names"]
    ]


def _run_device(concat_in):
    """concat_in: list of global (8*dim0, ...) arrays (np or jax). Returns
    list of per-core output dicts."""
    st = _get_exec()
    out_arrs = st["sharded"](*concat_in, *st["zeros_dev"])
    res = []
    for c in range(N_CORES):
        d = {}
        for i, name in enumerate(st["out_names"]):
            shp = st["out_avals"][i].shape
            d[name] = np.asarray(out_arrs[i]).reshape(N_CORES, *shp)[c]
        res.append(d)
    return res


def kernel(x, w_qkv, b_qkv, w_proj, b_proj):
    x = np.asarray(x, dtype=np.float32)
    w_qkv = np.asarray(w_qkv, dtype=np.float32)
    b_qkv = np.asarray(b_qkv, dtype=np.float32)
    w_proj = np.asarray(w_proj, dtype=np.float32)
    b_proj = np.asarray(b_proj, dtype=np.float32)

    in_maps = _make_in_maps(x, w_qkv, b_qkv)
    for c in range(N_CORES):
        _, hg = divmod(c, HPC)
        j0 = DJ * hg
        in_maps[c]["wp"] = np.ascontiguousarray(
            w_proj[j0 : j0 + DJ, :].astype(np.float16)
        )

    results = _run_device(_concat_inputs(in_maps))

    out = np.zeros((B, T, E), dtype=np.float32)
    for c in range(N_CORES):
        out[c // HPC] += np.asarray(results[c]["out"], dtype=np.float32)
    # fold b_v through the projection; b_k cancels inside softmax
    bias = b_proj + b_qkv[2 * E :] @ w_proj
    out += bias[None, None, :]
    return out
